# revision 1
# baseline (speedup 1.0000x reference)
"""PointWarping v2: fp16 score selection (2x DVE) + exact host re-rank.

Device per core: augmented matmul scores (f32 PSUM) are cast to fp16 on
the PSUM->SBUF copy; DVE max / max_index run at 2x 16-bit throughput and
return the top-8 candidate values+indices per query.  Host re-ranks the
8 candidates with exact f32 distances (reference formula), computes the
weights, gathers neighbor flows and warps.  Queries where the fp16
3rd==8th value ties (candidate set not provably complete) or duplicate
indices appear are recomputed exactly on host (rare).
"""

import numpy as np

B, C, N = 4, 3, 8192
NQ = 4096
NT = 32
EPS = 1e-10
CLAMP = 10.0

_CACHE = {}


def _build():
    if "nc" in _CACHE:
        return _CACHE["nc"]

    from contextlib import ExitStack
    from concourse import bacc, bass, tile
    from concourse import mybir

    nc = bacc.Bacc("TRN2", target_bir_lowering=False, debug=False,
                   enable_asserts=True, num_devices=1)
    f32 = mybir.dt.float32
    f32r = mybir.dt.float32r
    f16 = mybir.dt.float16
    i16 = mybir.dt.int16
    u32 = mybir.dt.uint32
    ADD = mybir.AluOpType.add
    MULT = mybir.AluOpType.mult

    q2 = nc.dram_tensor("q2", [3, NQ], f32, kind="ExternalInput").ap()
    p1 = nc.dram_tensor("p1", [3, N], f32, kind="ExternalInput").ap()
    f1 = nc.dram_tensor("f1", [3, N], f32, kind="ExternalInput").ap()
    p1b = nc.dram_tensor("p1b", [32, 768], f32, kind="ExternalInput").ap()
    f1b = nc.dram_tensor("f1b", [32, 768], f32, kind="ExternalInput").ap()
    vallo = nc.dram_tensor("vallo", [128, 8 * NT], f32,
                           kind="ExternalOutput").ap()
    gidxo = nc.dram_tensor("gidxo", [128, 8 * NT], i16,
                           kind="ExternalOutput").ap()

    with tile.TileContext(nc) as tc, ExitStack() as ctx:
        cp = ctx.enter_context(tc.tile_pool(name="persist", bufs=1))
        spool = ctx.enter_context(tc.tile_pool(name="scores", bufs=2))
        ppool = ctx.enter_context(tc.tile_pool(name="ps", bufs=2, space="PSUM"))
        tp = ctx.enter_context(tc.tile_pool(name="loop", bufs=2))

        def pt(shape, dtype=f32, tag=None):
            return cp.tile(shape, dtype, tag=tag, bufs=1, name=tag or "ptile")

        QSTG = spool.tile([4, NQ], f32, tag="S", name="QSTG")
        nc.vector.memset(QSTG[:, :], -1.0)
        nc.sync.dma_start(QSTG[0:3, :], q2[:, :])
        nc.vector.tensor_scalar(QSTG[0:3, :], QSTG[0:3, :], 2.0, None, MULT)
        QAUG = pt([4, NQ], f32r, tag="QAUG")
        nc.gpsimd.tensor_copy(QAUG[:], QSTG[:])

        KSTG = spool.tile([4, N], f32, tag="S", name="KSTG")
        F1T = pt([3, N], tag="F1T")
        nc.sync.dma_start(KSTG[0:3, :], p1[:, :])
        nc.sync.dma_start(F1T[:], f1[:, :])
        nc.vector.tensor_tensor(KSTG[0:3, :], KSTG[0:3, :], F1T[:], ADD)
        P1B = pt([32, 768], tag="P1B")
        F1B = pt([32, 768], tag="F1B")
        nc.sync.dma_start(P1B[:], p1b[:, :])
        nc.sync.dma_start(F1B[:], f1b[:, :])
        KSQ = pt([32, 768], tag="KSQ")
        nc.vector.tensor_tensor(KSQ[:], P1B[:], F1B[:], ADD)
        nc.scalar.square(KSQ[:], KSQ[:])
        NORM = pt([32, 256], tag="NORM")
        nc.vector.tensor_tensor(NORM[:], KSQ[:, 0:256], KSQ[:, 256:512], ADD)
        nc.vector.tensor_tensor(NORM[:], NORM[:], KSQ[:, 512:768], ADD)
        nc.sync.dma_start(KSTG[3:4, :], NORM[:])
        KAUG = pt([4, N], f32r, tag="KAUG")
        nc.gpsimd.tensor_copy(KAUG[:], KSTG[:])

        VAL8 = pt([128, 8 * NT], tag="VAL8")        # top-8 fp16 scores (as f32)
        GIDX8 = pt([128, 8 * NT], i16, tag="GIDX8")  # top-8 indices

        for t in range(NT):
            S = spool.tile([128, N], f16, tag="S", name="S")
            lhsT = QAUG[:, bass.ts(t, 128)]
            for kc in range(4):
                P = ppool.tile([128, 2048], f32, tag="P", name="P")
                for i in range(4):
                    nc.tensor.matmul(
                        P[:, bass.ts(i, 512)],
                        lhsT,
                        KAUG[:, 2048 * kc + 512 * i:2048 * kc + 512 * (i + 1)],
                        start=True, stop=True)
                nc.scalar.copy(S[:, bass.ts(kc, 2048)], P[:])
            V8 = tp.tile([128, 8], f16, tag="V8", name="V8")
            nc.vector.max(V8[:], S[:])
            I8 = tp.tile([128, 8], u32, tag="I8", name="I8")
            nc.vector.max_index(I8[:], V8[:], S[:])
            nc.gpsimd.tensor_copy(VAL8[:, 8 * t:8 * t + 8], V8[:])
            nc.gpsimd.tensor_copy(GIDX8[:, 8 * t:8 * t + 8], I8[:])

        nc.sync.dma_start(vallo[:, :], VAL8[:])
        nc.sync.dma_start(gidxo[:, :], GIDX8[:])

    nc.compile()
    _CACHE["nc"] = nc
    return nc


def make_core_inputs(pos1, pos2, flow1, core):
    b, h = core // 2, core % 2
    q2 = np.ascontiguousarray(pos2[b, :, h * NQ:(h + 1) * NQ])
    p1 = np.ascontiguousarray(pos1[b])
    f1 = np.ascontiguousarray(flow1[b])
    p1b = np.ascontiguousarray(
        pos1[b].reshape(3, 32, 256).transpose(1, 0, 2).reshape(32, 768))
    f1b = np.ascontiguousarray(
        flow1[b].reshape(3, 32, 256).transpose(1, 0, 2).reshape(32, 768))
    return {"q2": q2, "p1": p1, "f1": f1, "p1b": p1b, "f1b": f1b}


def combine_host(q2, pos1b, flow1b, val8, gidx8):
    """Exact re-rank of device top-8 candidates + weighted warp.

    q2 [3, NQ] queries for this core; pos1b/flow1b [3, 8192];
    val8/gidx8 [128, 8*NT] device outputs (query (t,p) -> row p, cols 8t..).
    Returns [C, NQ] (column q = 128t + p).
    """
    q = np.ascontiguousarray(
        q2.reshape(3, NT, 128).transpose(2, 1, 0)).astype(np.float32)
    idx = np.asarray(gidx8).astype(np.int64).reshape(128, NT, 8)
    v = np.asarray(val8, dtype=np.float32).reshape(128, NT, 8)
    k = (pos1b + flow1b).T.astype(np.float32)            # [8192, 3]
    fl = flow1b.T.astype(np.float32)                     # [8192, 3]

    diff = k[idx] - q[:, :, None, :]                     # [p,t,8,3]
    d2c = (diff * diff).sum(-1, dtype=np.float32)        # [p,t,8]
    order = np.lexsort((idx, d2c), axis=-1)[..., :3]     # by d2 then index
    i3 = np.take_along_axis(idx, order, -1)              # [p,t,3]
    d2_3 = np.take_along_axis(d2c, order, -1)

    # fp16 v3 == v8  =>  candidate set may be incomplete; dup indices too
    vh = v.astype(np.float16)
    flag = vh[..., 2] == vh[..., 7]
    si = np.sort(idx, axis=-1)
    flag |= (np.diff(si, axis=-1) == 0).any(-1)
    if flag.any():
        fp, ft = np.nonzero(flag)
        qf = q[fp, ft]                                   # [m,3]
        d2f = ((qf[:, None, :] - k[None, :, :]) ** 2).sum(-1, dtype=np.float32)
        of = np.argsort(d2f, axis=1, kind="stable")[:, :3]
        i3[fp, ft] = of
        d2_3[fp, ft] = np.take_along_axis(d2f, of, 1)

    dist = np.maximum(np.sqrt(np.maximum(d2_3, 0.0)), EPS).astype(np.float32)
    inv = (1.0 / dist).astype(np.float32)
    w = inv / inv.sum(-1, keepdims=True)
    flow2 = (w[..., None] * fl[i3]).sum(-2, dtype=np.float32)  # [p,t,3]
    res = q - flow2
    np.clip(res, -CLAMP, CLAMP, out=res)
    return res.transpose(2, 1, 0).reshape(C, NQ)


def kernel(pos1, pos2, flow1):
    from concourse.bass_utils import run_bass_kernel_spmd

    pos1 = np.asarray(pos1, dtype=np.float32)
    pos2 = np.asarray(pos2, dtype=np.float32)
    flow1 = np.asarray(flow1, dtype=np.float32)

    nc = _build()
    in_maps = [make_core_inputs(pos1, pos2, flow1, c) for c in range(8)]
    res = run_bass_kernel_spmd(nc, in_maps, core_ids=list(range(8)))

    full = np.empty((B, C, N), dtype=np.float32)
    for core in range(8):
        b, h = core // 2, core % 2
        full[b, :, h * NQ:(h + 1) * NQ] = combine_host(
            in_maps[core]["q2"], pos1[b], flow1[b],
            res.results[core]["vallo"], res.results[core]["gidxo"])
    return full



# revision 16
# speedup vs baseline: 2.2664x; 2.2664x over previous
"""PointWarping v3: block-winnow device kernel + exact host re-rank.

Device per core (4096 queries x 8192 points, 32 tiles of 128 queries):
augmented f32r matmul writes exact scores s = 2q.k - |k|^2 into 8 PSUM
chunks of [128,1024]; the Pool engine max-merges two chunk pairs straight
from PSUM, ACT casts the other four chunks to fp16, and DVE folds a fp16
max tree down to M4[512] = per-block max over 512 blocks of 16 columns
(block b = cols == b mod 512).  DVE max/max_index emit the top-8 block
ids per query plus the 8th block value.  Host gathers the 8x16 = 128
candidate points per query (provably a superset of the true 3-NN unless
the row is flagged), re-ranks exactly in jitted jax CPU, and computes the
inverse-distance-weighted warp.  Flagged rows (duplicate block ids from
fp16 value ties, or 3rd-candidate score not strictly above the 8th block
value) are recomputed exactly on host - rare.

Execution uses a cached jit of the shard_map body (no per-call retrace)
and per-shard async device-to-host copies (single tunnel round trip).
"""

import numpy as np

B, C, N = 4, 3, 8192
NQ = 4096
NT = 32
NBLK = 512          # blocks per query row
BLKW = 16           # columns per block (stride NBLK)
EPS = 1e-10
CLAMP = 10.0

_CACHE = {}


def _build():
    if "nc" in _CACHE:
        return _CACHE["nc"]

    from contextlib import ExitStack
    from concourse import bacc, bass, tile
    from concourse import mybir

    nc = bacc.Bacc("TRN2", target_bir_lowering=False, debug=False,
                   enable_asserts=True, num_devices=1)
    f32 = mybir.dt.float32
    f32r = mybir.dt.float32r
    f16 = mybir.dt.float16
    u16 = mybir.dt.uint16
    MAX = mybir.AluOpType.max

    qaug = nc.dram_tensor("qaug", [4, NQ], f32r, kind="ExternalInput").ap()
    kaug = nc.dram_tensor("kaug", [4, N], f32r, kind="ExternalInput").ap()
    vallo = nc.dram_tensor("vallo", [128, 8 * NT], f16,
                           kind="ExternalOutput").ap()
    gidxo = nc.dram_tensor("gidxo", [128, 8 * NT], u16,
                           kind="ExternalOutput").ap()

    with tile.TileContext(nc) as tc, ExitStack() as ctx:
        cp = ctx.enter_context(tc.tile_pool(name="persist", bufs=1))
        tp = ctx.enter_context(tc.tile_pool(name="loop", bufs=3))
        pp = ctx.enter_context(tc.tile_pool(name="ps", bufs=4, space="PSUM"))

        QAUG = cp.tile([4, NQ], f32r, tag="QAUG", bufs=1, name="QAUG")
        KAUG = cp.tile([4, N], f32r, tag="KAUG", bufs=1, name="KAUG")
        # split the big prologue loads across parallel DMA queues
        nc.sync.dma_start(QAUG[:, :], qaug[:, :])
        nc.scalar.dma_start(KAUG[:, 0:4096], kaug[:, 0:4096])
        nc.gpsimd.dma_start(KAUG[:, 4096:8192], kaug[:, 4096:8192])

        VAL8 = cp.tile([128, 8 * NT], f16, tag="VAL8", bufs=1, name="VAL8")
        GIDX8 = cp.tile([128, 8 * NT], u16, tag="GIDX8", bufs=1, name="GIDX8")

        # software-pipelined tail: M4/max8/max_index of tile t-1 are emitted
        # during tile t so late-chain deps never block the next tile's work
        pend = None

        def emit_tail(pM, pt):
            M4 = tp.tile([128, 512], f16, tag="M4", name="M4")
            nc.vector.tensor_tensor(M4[:, :], pM[:, 0:512], pM[:, 512:1024],
                                    MAX)
            BV = VAL8[:, 8 * pt:8 * pt + 8]
            nc.vector.max(BV, M4[:, :])
            nc.vector.max_index(GIDX8[:, 8 * pt:8 * pt + 8], BV, M4[:, :])

        for t in range(NT):
            lhsT = QAUG[:, bass.ts(t, 128)]
            # 8 PSUM chunks of [128, 1024]; chunk k covers cols [1024k, +1024)
            ch = []
            for k in range(8):
                P = pp.tile([128, 1024], f32, tag="P", bufs=4, name=f"P{k}")
                for i in range(2):
                    nc.tensor.matmul(
                        P[:, bass.ts(i, 512)], lhsT,
                        KAUG[:, 1024 * k + 512 * i:1024 * k + 512 * (i + 1)],
                        start=True, stop=True)
                ch.append(P)

            # drain PSUM: only ACT (casts) and single-PSUM-operand DVE maxes
            # may read it; the Pool engine supports neither PSUM nor max.
            # Alternate 6/7 ACT casts per tile to balance ACT vs DVE.
            def cast(k):
                A = tp.tile([128, 1024], f16, tag=f"A{k}", name=f"A{k}")
                nc.scalar.copy(A[:, :], ch[k][:, :])
                return A

            def tt(x, y, tag):
                Z = tp.tile([128, 1024], f16, tag=tag, name=tag)
                nc.vector.tensor_tensor(Z[:, :], x[:, :], y[:, :], MAX)
                return Z

            M = tp.tile([128, 1024], f16, tag="M", name="M")
            if t % 2 == 0:
                # casts c0,c1,c4,c5,c2,c3 (c4/c5 early: they gate PSUM
                # slot reuse); DVE drains c6,c7 fused with A0/A1
                A0, A1 = cast(0), cast(1)
                A4, A5 = cast(4), cast(5)
                A2, A3 = cast(2), cast(3)
                R1 = tt(ch[6], A0, "R1")
                R2 = tt(ch[7], A1, "R2")
                m1 = tt(A2, A3, "m1")
                m2 = tt(A4, A5, "m2")
                m3 = tt(R1, R2, "m3")
                m4 = tt(m1, m2, "m4")
                nc.vector.tensor_tensor(M[:, :], m3[:, :], m4[:, :], MAX)
            else:
                # casts c0,c4,c5,c6,c1,c2,c3; DVE drains c7 fused with A0
                A0 = cast(0)
                A4, A5, A6 = cast(4), cast(5), cast(6)
                A1, A2, A3 = cast(1), cast(2), cast(3)
                R1 = tt(ch[7], A0, "R1")
                m1 = tt(A1, A2, "m1")
                m2 = tt(A3, A4, "m2")
                m3 = tt(A5, A6, "m3")
                m4 = tt(m1, m2, "m4")
                m5 = tt(m3, R1, "m5")
                nc.vector.tensor_tensor(M[:, :], m4[:, :], m5[:, :], MAX)

            if pend is not None:
                emit_tail(*pend)
            pend = (M, t)

        emit_tail(*pend)

        nc.sync.dma_start(vallo[:, :], VAL8[:, :])
        nc.sync.dma_start(gidxo[:, :], GIDX8[:, :])

    nc.compile()
    _CACHE["nc"] = nc
    return nc


def _get_runner():
    if "runner" in _CACHE:
        return _CACHE["runner"]

    import jax
    from jax.sharding import Mesh, PartitionSpec
    import warnings
    with warnings.catch_warnings():
        warnings.simplefilter("ignore")
        try:
            from jax.experimental.shard_map import shard_map
        except ImportError:
            from jax import shard_map
    from concourse import mybir
    from concourse.bass2jax import (
        install_neuronx_cc_hook,
        _bass_exec_p,
        partition_id_tensor,
    )

    nc = _build()
    n_cores = 8
    install_neuronx_cc_hook()
    partition_name = (nc.partition_id_tensor.name
                      if nc.partition_id_tensor else None)

    in_names, out_names, out_avals, zero_outs = [], [], [], []
    for alloc in nc.m.functions[0].allocations:
        if not isinstance(alloc, mybir.MemoryLocationSet):
            continue
        name = alloc.memorylocations[0].name
        if alloc.kind == "ExternalInput":
            if name != partition_name:
                in_names.append(name)
        elif alloc.kind == "ExternalOutput":
            out_names.append(name)
            shape = tuple(alloc.tensor_shape)
            dtype = mybir.dt.np(alloc.dtype)
            out_avals.append(jax.core.ShapedArray(shape, dtype))
            zero_outs.append((shape, dtype))
    n_params = len(in_names)
    n_outs = len(out_avals)
    all_names = list(in_names) + list(out_names)
    if partition_name is not None:
        all_names.append(partition_name)

    donate = tuple(range(n_params, n_params + n_outs))

    def _body(*args):
        operands = list(args)
        if partition_name is not None:
            operands.append(partition_id_tensor())
        outs = _bass_exec_p.bind(
            *operands,
            out_avals=tuple(out_avals),
            in_names=tuple(all_names),
            out_names=tuple(out_names),
            lowering_input_output_aliases=(),
            sim_require_finite=True,
            sim_require_nnan=True,
            nc=nc,
        )
        return tuple(outs)

    devices = jax.devices()[:n_cores]
    mesh = Mesh(np.asarray(devices), ("core",))
    in_specs = (PartitionSpec("core"),) * (n_params + n_outs)
    out_specs = (PartitionSpec("core"),) * len(out_names)
    try:
        smapped = shard_map(_body, mesh=mesh, in_specs=in_specs,
                            out_specs=out_specs, check_vma=False)
    except TypeError:
        smapped = shard_map(_body, mesh=mesh, in_specs=in_specs,
                            out_specs=out_specs, check_rep=False)
    sharded = jax.jit(smapped, donate_argnums=donate, keep_unused=True)

    runner = {
        "sharded": sharded,
        "in_names": in_names,
        "out_names": out_names,
        "zero_outs": zero_outs,
        "n_cores": n_cores,
    }
    _CACHE["runner"] = runner
    return runner


def _run_device(in_maps):
    """Run the bass kernel on 8 cores; returns per-core output dicts."""
    import jax

    r = _get_runner()
    n_cores = r["n_cores"]
    concat_in = [
        np.concatenate([np.asarray(m[name]) for m in in_maps], axis=0)
        for name in r["in_names"]
    ]
    concat_zeros = [
        np.zeros((n_cores * s[0], *s[1:]), d) for s, d in r["zero_outs"]
    ]
    out = r["sharded"](*concat_in, *concat_zeros)
    for a in out:
        for sh in a.addressable_shards:
            sh.data.copy_to_host_async()
    res = [np.asarray(a) for a in out]
    return [
        {name: res[i].reshape(n_cores, *r["zero_outs"][i][0])[c]
         for i, name in enumerate(r["out_names"])}
        for c in range(n_cores)
    ]


def make_core_inputs(pos1, pos2, flow1, core):
    b, h = core // 2, core % 2
    q = pos2[b, :, h * NQ:(h + 1) * NQ]
    qaug = np.empty((4, NQ), np.float32)
    qaug[0:3] = 2.0 * q
    qaug[3] = -1.0
    k = pos1[b] + flow1[b]
    kaug = np.empty((4, N), np.float32)
    kaug[0:3] = k
    kaug[3] = (k * k).sum(axis=0)
    return {"qaug": qaug, "kaug": kaug}


def _get_combine():
    if "combine" in _CACHE:
        return _CACHE["combine"]

    import jax
    import jax.numpy as jnp

    def _one(q, kdb, flow, blk, bval):
        # q [3, NQ]; kdb/flow [N, 3]; blk [128, 8*NT] i32; bval [128, 8*NT] f16
        qc = q.reshape(3, NT, 128).transpose(2, 1, 0)        # [p, t, 3]
        blk = blk.reshape(128, NT, 8)
        cand = (blk[..., None] + NBLK * jnp.arange(BLKW, dtype=blk.dtype))
        cand = cand.reshape(128, NT, 8 * BLKW)               # [p, t, 128]
        kc = jnp.take(kdb, cand, axis=0)                     # [p, t, 128, 3]
        diff = kc - qc[:, :, None, :]
        d2 = jnp.sum(diff * diff, axis=-1)                   # [p, t, 128]
        negd3, pos3 = jax.lax.top_k(-d2, 3)
        d3 = -negd3                                          # [p, t, 3] ascending
        i3 = jnp.take_along_axis(cand, pos3, axis=-1)        # point indices
        dist = jnp.maximum(jnp.sqrt(jnp.maximum(d3, 0.0)), EPS)
        inv = 1.0 / dist
        w = inv / jnp.sum(inv, axis=-1, keepdims=True)
        f3 = jnp.take(flow, i3, axis=0)                      # [p, t, 3, 3]
        flow2 = jnp.sum(w[..., None] * f3, axis=-2)          # [p, t, 3]
        res = jnp.clip(qc - flow2, -CLAMP, CLAMP)
        out = res.transpose(2, 1, 0).reshape(3, NQ)

        # flags: duplicate block ids (fp16 value ties in max_index), or the
        # 3rd candidate's score not strictly above the 8th block value
        sb = jnp.sort(blk, axis=-1)
        dup = jnp.any(sb[..., 1:] == sb[..., :-1], axis=-1)  # [p, t]
        qn = jnp.sum(qc * qc, axis=-1)                       # [p, t]
        s3c = (qn - d3[..., 2]).astype(jnp.float16)
        bval7 = bval.reshape(128, NT, 8)[..., 7]             # 8th block value
        flag = dup | (s3c <= bval7)
        return out, flag

    fn = jax.jit(jax.vmap(_one))
    _CACHE["combine"] = fn
    return fn


def _fallback_exact(q, kdb, flow, rows):
    """Exact 3-NN warp for flagged query rows. rows: (p_idx, t_idx)."""
    p_idx, t_idx = rows
    cols = 128 * t_idx + p_idx
    qf = q[:, cols].T                                        # [m, 3]
    d2 = ((qf[:, None, :] - kdb[None, :, :]) ** 2).sum(-1, dtype=np.float32)
    order = np.argsort(d2, axis=1, kind="stable")[:, :3]
    d3 = np.take_along_axis(d2, order, 1)
    dist = np.maximum(np.sqrt(np.maximum(d3, 0.0)), EPS)
    inv = 1.0 / dist
    w = inv / inv.sum(-1, keepdims=True)
    flow2 = (w[..., None] * flow[order]).sum(-2, dtype=np.float32)
    return cols, np.clip(qf - flow2, -CLAMP, CLAMP)          # [m, 3]


def kernel(pos1, pos2, flow1):
    import jax

    pos1 = np.asarray(pos1, dtype=np.float32)
    pos2 = np.asarray(pos2, dtype=np.float32)
    flow1 = np.asarray(flow1, dtype=np.float32)

    in_maps = [make_core_inputs(pos1, pos2, flow1, c) for c in range(8)]
    outs = _run_device(in_maps)

    q_all = np.stack([pos2[c // 2, :, (c % 2) * NQ:(c % 2 + 1) * NQ]
                      for c in range(8)])
    kdb_all = np.stack([
        np.ascontiguousarray(in_maps[c]["kaug"][0:3].T) for c in range(8)
    ])
    flow_all = np.stack([
        np.ascontiguousarray(flow1[c // 2].T) for c in range(8)
    ])
    blk_all = np.stack([outs[c]["gidxo"].astype(np.int32) for c in range(8)])
    bval_all = np.stack([outs[c]["vallo"] for c in range(8)])

    cpu = jax.devices("cpu")[0]
    with jax.default_device(cpu):
        res, flag = _get_combine()(q_all, kdb_all, flow_all, blk_all, bval_all)
    res = np.asarray(res)
    flag = np.asarray(flag)

    full = np.empty((B, C, N), dtype=np.float32)
    for c in range(8):
        b, h = c // 2, c % 2
        out_c = res[c]
        if flag[c].any():
            rows = np.nonzero(flag[c])
            cols, fixed = _fallback_exact(
                q_all[c], kdb_all[c], flow_all[c], rows)
            out_c = out_c.copy()
            out_c[:, cols] = fixed.T
        full[b, :, h * NQ:(h + 1) * NQ] = out_c
    return full
